# revision 1
# baseline (speedup 1.0000x reference)
"""Local multi-headed attention (window +/-2) + residual + LayerNorm, Trainium2 Bass kernel.

Sharding: data-parallel over batch. B=8 batch elements -> one per NeuronCore (8 cores).
Each core computes the full sequence for its batch element; no collectives.

v3 design (engine-balanced):
  - CHUNK=512 sequence positions per pipeline step (8 chunks).
  - x loaded [s,d], PE-transposed to xT [d,s] bf16; Q/K/V projections W.T @ xT with
    fp32 PSUM accumulate; PSUM->SBUF copy fused with per-partition bias on ScalarE.
  - Scores in REPLICATED-COMPACT layout: per d-tile, prod = qT * shift_w(kT) (DVE
    bf16 2x), then matmuls with selector stationaries [128,96] accumulate over all
    6 d-tiles into PSUM rows 12k+h (k=0..7 replicas of head h). Replication is free
    on the PE (cost ~ moving free size) and softmax engine time is free too
    (ScalarE/DVE time scales with free dim, not partitions). The replicas exist so
    the later broadcast DMA's *source* partitions span many SDMA engine groups
    (descriptor->engine assignment keys on source partition).
  - Softmax compact: one EXP per half-chunk on ScalarE, denominator = 4 DVE adds,
    reciprocal_approx_fast, normalization folded into the compact weights.
  - exn broadcast to the [128 = 2 heads x 64 dims] layout per d-tile via SBUF->SBUF
    DMA: dst partition p <- src row 12*(p//8 % 8) + (2dt + p//64).
  - AV: avp_w = exn_bc * shift_w(vT) and add tree, all DVE bf16 2x.
  - O-projection: att blocks stationary vs Wo bf16, bias via K=1 ones-row matmul,
    PSUM from the shared [128,512] pool (two tiles per s-tile).
  - Residual+LayerNorm: ypre on DVE (PSUM+SBUF), stats via bn_stats/bn_aggr,
    rstd = exp(-0.5*ln(var+eps)) batched per chunk (one Ln + one Exp for 4 s-tiles)
    to bound ACT_TABLE_LOAD thrash; gamma/beta application on GpSimd.
"""
import numpy as np

B, S, D = 8, 4096, 768
HEADS = 12
DH = 64
W = 5            # window taps, offsets -2..2
CHUNK = 512      # sequence chunk per pipeline step
NCH = S // CHUNK
HALF = 256       # scores/softmax half-chunk granularity (PSUM bank budget)
DT = D // 128    # 6 partition tiles of d
REP = 8          # compact-score replicas (source spread for broadcast DMA)
CROWS = REP * HEADS  # 96 compact rows
EPS = 1e-5
N_CORES = 8

_cache = {}


def _build():
    import concourse.bass as bass
    import concourse.tile as tile
    from concourse import bacc, mybir
    from concourse.masks import make_identity

    f32 = mybir.dt.float32
    bf16 = mybir.dt.bfloat16
    AF = mybir.ActivationFunctionType
    ALU = mybir.AluOpType

    nc = bacc.Bacc("TRN2", target_bir_lowering=False, debug=False,
                   num_devices=N_CORES)

    x_ap = nc.dram_tensor("x", [S, D], f32, kind="ExternalInput").ap()
    wq_ap = nc.dram_tensor("Wq", [D, D], f32, kind="ExternalInput").ap()
    bq_ap = nc.dram_tensor("bq", [D], f32, kind="ExternalInput").ap()
    wk_ap = nc.dram_tensor("Wk", [D, D], f32, kind="ExternalInput").ap()
    bk_ap = nc.dram_tensor("bk", [D], f32, kind="ExternalInput").ap()
    wv_ap = nc.dram_tensor("Wv", [D, D], f32, kind="ExternalInput").ap()
    bv_ap = nc.dram_tensor("bv", [D], f32, kind="ExternalInput").ap()
    wo_ap = nc.dram_tensor("Wo", [D, D], f32, kind="ExternalInput").ap()
    bo_ap = nc.dram_tensor("bo", [D], f32, kind="ExternalInput").ap()
    gamma_ap = nc.dram_tensor("gamma", [D], f32, kind="ExternalInput").ap()
    beta_ap = nc.dram_tensor("beta", [D], f32, kind="ExternalInput").ap()
    out_ap = nc.dram_tensor("out", [S, D], f32, kind="ExternalOutput").ap()

    with tile.TileContext(nc) as tc:
        with tc.tile_pool(name="persist", bufs=1) as pp:
            wq_sb = pp.tile([128, DT, D], bf16, tag="wq")
            wk_sb = pp.tile([128, DT, D], bf16, tag="wk")
            wv_sb = pp.tile([128, DT, D], bf16, tag="wv")
            wo_sb = pp.tile([128, DT, D], bf16, tag="wo")
            bqT = pp.tile([128, DT], f32, tag="bqT")
            bkT = pp.tile([128, DT], f32, tag="bkT")
            bvT = pp.tile([128, DT], f32, tag="bvT")
            gb_sb = pp.tile([1, D], f32, tag="g")
            be_sb = pp.tile([1, D], f32, tag="be")
            bo_sb = pp.tile([1, D], f32, tag="bo")
            ones_row = pp.tile([1, 128], f32, tag="ones")
            ones_bf = pp.tile([1, 128], bf16, tag="onesbf")
            bo_bf = pp.tile([1, D], bf16, tag="bobf")
            ident = pp.tile([128, 128], f32, tag="ident")
            gb_bc = pp.tile([128, D], bf16, tag="gbbc")
            be_bc = pp.tile([128, D], bf16, tag="bebc")
            sel = pp.tile([128, DT, CROWS], bf16, tag="sel")
            eps_sb = pp.tile([128, 1], f32, tag="eps")

            nc.vector.memset(ones_row[:], 1.0)
            nc.vector.memset(eps_sb[:], EPS)
            nc.vector.memset(ones_bf[:], 1.0)
            make_identity(nc, ident[:])
            # selector stationaries: sel[:, dt, :] maps prod partitions (2 heads
            # of 64 dims) onto compact rows 12k + h (8 replicas per head,
            # interleaved layout -> broadcast DMA sources span SDMA engines)
            nc.vector.memset(sel[:], 0.0)
            for dt in range(DT):
                for k in range(REP):
                    nc.vector.memset(
                        sel[0:64, dt, 12 * k + 2 * dt:12 * k + 2 * dt + 1], 1.0)
                    nc.vector.memset(
                        sel[64:128, dt,
                            12 * k + 2 * dt + 1:12 * k + 2 * dt + 2], 1.0)

            nc.sync.dma_start(bqT[:], bq_ap.rearrange("(t p) -> p t", p=128))
            nc.sync.dma_start(bkT[:], bk_ap.rearrange("(t p) -> p t", p=128))
            nc.sync.dma_start(bvT[:], bv_ap.rearrange("(t p) -> p t", p=128))
            nc.sync.dma_start(bo_sb[:], bo_ap[:])
            nc.sync.dma_start(gb_sb[:], gamma_ap[:])
            nc.sync.dma_start(be_sb[:], beta_ap[:])
            nc.vector.tensor_copy(bo_bf[:], bo_sb[:])

            with tc.tile_pool(name="wstage", bufs=3) as wsp:
                for w_ap, sb in ((wq_ap, wq_sb), (wk_ap, wk_sb),
                                 (wv_ap, wv_sb), (wo_ap, wo_sb)):
                    for half in range(3):
                        st = wsp.tile([128, 2, D], f32, tag="wstage")
                        nc.sync.dma_start(
                            st[:],
                            w_ap[half * 256:(half + 1) * 256, :].rearrange(
                                "(kt p) n -> p kt n", p=128))
                        nc.vector.tensor_copy(sb[:, 2 * half:2 * half + 2, :],
                                              st[:])

            with tc.tile_pool(name="initps", bufs=1, space="PSUM") as initps:
                for src, dst in ((gb_sb, gb_bc), (be_sb, be_bc)):
                    t = initps.tile([128, D], f32, tag="gbps")
                    nc.tensor.matmul(t[:, 0:512], ones_row[:], src[:, 0:512])
                    nc.tensor.matmul(t[:, 512:D], ones_row[:], src[:, 512:D])
                    nc.vector.tensor_copy(dst[:], t[:])

            with tc.tile_pool(name="ppsum", bufs=5, space="PSUM") as ppsum, \
                 tc.tile_pool(name="scps", bufs=1, space="PSUM") as scps, \
                 tc.tile_pool(name="xpool", bufs=3) as xpool, \
                 tc.tile_pool(name="xtpool", bufs=1) as xtpool, \
                 tc.tile_pool(name="qpool", bufs=2) as qpool, \
                 tc.tile_pool(name="kvpool", bufs=2) as kvpool, \
                 tc.tile_pool(name="prpool", bufs=2) as prpool, \
                 tc.tile_pool(name="expool", bufs=2) as expool, \
                 tc.tile_pool(name="enpool", bufs=2) as enpool, \
                 tc.tile_pool(name="exbpool", bufs=2) as exbpool, \
                 tc.tile_pool(name="avpool", bufs=2) as avpool, \
                 tc.tile_pool(name="atpool", bufs=2) as atpool, \
                 tc.tile_pool(name="yppool", bufs=4) as yppool, \
                 tc.tile_pool(name="ypool", bufs=2) as ypool, \
                 tc.tile_pool(name="stpool", bufs=2) as stpool:

                kc_tiles = [None] * NCH
                vc_tiles = [None] * NCH

                def load_x(c):
                    s0 = c * CHUNK
                    x_sb = xpool.tile([128, 4, D], f32, tag="x")
                    nc.scalar.dma_start(
                        x_sb[:], x_ap[s0:s0 + CHUNK, :].rearrange(
                            "(st p) d -> p st d", p=128))
                    return x_sb

                def project(c, x_sb):
                    xT = xtpool.tile([128, DT, CHUNK], bf16, tag="xT")
                    for dt in range(DT):
                        tp = ppsum.tile([128, CHUNK], f32, tag="ps")
                        for st in range(4):
                            nc.tensor.transpose(
                                tp[:, st * 128:(st + 1) * 128],
                                x_sb[:, st, dt * 128:(dt + 1) * 128], ident[:])
                        nc.scalar.copy(xT[:, dt, :], tp[:])

                    qT = qpool.tile([128, DT, CHUNK], bf16, tag="qT")
                    kc = kvpool.tile([128, DT, CHUNK + 4], bf16, tag="kc")
                    vc = kvpool.tile([128, DT, CHUNK + 4], bf16, tag="vc")
                    kc_tiles[c] = kc
                    vc_tiles[c] = vc
                    for (wsb, bT, dst, off) in ((wq_sb, bqT, qT, None),
                                                (wk_sb, bkT, kc, 2),
                                                (wv_sb, bvT, vc, 2)):
                        for dt in range(DT):
                            ps = ppsum.tile([128, CHUNK], f32, tag="ps")
                            for kt in range(DT):
                                nc.tensor.matmul(
                                    ps[:],
                                    wsb[:, kt, dt * 128:(dt + 1) * 128],
                                    xT[:, kt, :],
                                    start=(kt == 0), stop=(kt == DT - 1))
                            dslice = dst[:, dt, :] if off is None \
                                else dst[:, dt, 2:2 + CHUNK]
                            nc.scalar.activation(dslice, ps[:], AF.Identity,
                                                 bias=bT[:, dt:dt + 1])
                    if c > 0:
                        for big_prev, big_cur in ((kc_tiles[c - 1], kc),
                                                  (vc_tiles[c - 1], vc)):
                            nc.vector.tensor_copy(
                                big_cur[:, :, 0:2],
                                big_prev[:, :, CHUNK:CHUNK + 2])
                            nc.vector.tensor_copy(
                                big_prev[:, :, CHUNK + 2:CHUNK + 4],
                                big_cur[:, :, 2:4])
                    if c == 0:
                        for big, bT in ((kc, bkT), (vc, bvT)):
                            for dt in range(DT):
                                nc.vector.memset(big[:, dt, 0:2], 0.0)
                                nc.scalar.activation(
                                    big[:, dt, 0:2], big[:, dt, 0:2],
                                    AF.Identity, bias=bT[:, dt:dt + 1])
                    if c == NCH - 1:
                        for big, bT in ((kc, bkT), (vc, bvT)):
                            for dt in range(DT):
                                nc.vector.memset(big[:, dt, CHUNK + 2:CHUNK + 4],
                                                 0.0)
                                nc.scalar.activation(
                                    big[:, dt, CHUNK + 2:CHUNK + 4],
                                    big[:, dt, CHUNK + 2:CHUNK + 4],
                                    AF.Identity, bias=bT[:, dt:dt + 1])
                    return qT

                def attn_begin(c):
                    att = atpool.tile([128, DT, CHUNK], bf16, tag="att")
                    exbs = [None] * (CHUNK // HALF)
                    return att, exbs

                def attn_sm(c, qT, state, hh):
                    """scores + softmax + broadcast-issue for half hh."""
                    kc = kc_tiles[c]
                    att, exbs = state
                    ho = hh * HALF
                    sc = scps.tile([CROWS, W, HALF], f32, tag="sc")
                    sc_flat = sc[:].rearrange("p w s -> p (w s)")
                    for dt in range(DT):
                        prod = prpool.tile([128, W, HALF], bf16, tag="prod")
                        for w in range(W):
                            nc.vector.tensor_tensor(
                                prod[:, w, :], qT[:, dt, ho:ho + HALF],
                                kc[:, dt, w + ho:w + ho + HALF], ALU.mult)
                        pr_flat = prod[:].rearrange("p w s -> p (w s)")
                        nc.tensor.matmul(sc_flat[:, 0:512],
                                         sel[:, dt, :], pr_flat[:, 0:512],
                                         start=(dt == 0), stop=(dt == DT - 1))
                        nc.tensor.matmul(sc_flat[:, 512:1024],
                                         sel[:, dt, :], pr_flat[:, 512:1024],
                                         start=(dt == 0), stop=(dt == DT - 1))
                        nc.tensor.matmul(sc_flat[:, 1024:1280],
                                         sel[:, dt, :], pr_flat[:, 1024:1280],
                                         start=(dt == 0), stop=(dt == DT - 1))
                    ex = expool.tile([CROWS, W, HALF], bf16, tag="ex")
                    nc.scalar.activation(ex[:], sc[:], AF.Exp, scale=0.125)
                    t01 = stpool.tile([CROWS, HALF], bf16, tag="t01")
                    t23 = stpool.tile([CROWS, HALF], bf16, tag="t23")
                    zf = stpool.tile([CROWS, HALF], f32, tag="zf")
                    nc.vector.tensor_tensor(t01[:], ex[:, 0, :], ex[:, 1, :],
                                            ALU.add)
                    nc.vector.tensor_tensor(t23[:], ex[:, 2, :], ex[:, 3, :],
                                            ALU.add)
                    nc.vector.tensor_tensor(t01[:], t01[:], t23[:], ALU.add)
                    nc.vector.tensor_tensor(zf[:], t01[:], ex[:, 4, :], ALU.add)
                    rinv = stpool.tile([CROWS, HALF], f32, tag="rinv")
                    nc.vector.reciprocal_approx_fast(rinv[:], zf[:])
                    rb = stpool.tile([CROWS, HALF], bf16, tag="rb")
                    nc.vector.tensor_copy(rb[:], rinv[:])
                    exn = enpool.tile([CROWS, W, HALF], bf16, tag="exn")
                    for w in range(W):
                        nc.vector.tensor_tensor(exn[:, w, :],
                                                ex[:, w, :], rb[:], ALU.mult)
                    # broadcast this half's normalized weights per d-tile:
                    # dst partition p = h*64 + k*8 + r reads row 12k + 2dt + h
                    exn_h = exn[:]
                    pstep = exn_h.ap[0][0]
                    exb6 = []
                    for dt in range(DT):
                        exb = exbpool.tile([128, W, HALF], bf16, tag="exb")
                        for h in range(2):
                            bc_ap = bass.AP(
                                tensor=exn_h.tensor,
                                offset=exn_h.offset + (2 * dt + h) * pstep,
                                ap=[[12 * pstep, REP], [0, 8], [1, W * HALF]],
                            )
                            nc.sync.dma_start(exb[64 * h:64 * (h + 1)], bc_ap)
                        exb6.append(exb)
                    exbs[hh] = exb6

                def attn_av(c, state, hh):
                    """AV products + add tree for half hh."""
                    vc = vc_tiles[c]
                    att, exbs = state
                    ho = hh * HALF
                    for dt in range(DT):
                        exb = exbs[hh][dt]
                        avp = avpool.tile([128, W, HALF], bf16, tag="avp")
                        for w in range(W):
                            nc.vector.tensor_tensor(
                                avp[:, w, :], exb[:, w, :],
                                vc[:, dt, w + ho:w + ho + HALF], ALU.mult)
                        a01 = avpool.tile([128, HALF], bf16, tag="a01")
                        a23 = avpool.tile([128, HALF], bf16, tag="a23")
                        att_s = att[:, dt, ho:ho + HALF]
                        nc.vector.tensor_tensor(a01[:], avp[:, 0, :],
                                                avp[:, 1, :], ALU.add)
                        nc.vector.tensor_tensor(a23[:], avp[:, 2, :],
                                                avp[:, 3, :], ALU.add)
                        nc.vector.tensor_tensor(a01[:], a01[:], a23[:], ALU.add)
                        nc.vector.tensor_tensor(att_s, a01[:], avp[:, 4, :],
                                                ALU.add)

                def attn_oln(c, x_sb, state):
                    s0 = c * CHUNK
                    att = state[0]
                    # ---- O-projection + residual + LN stats per s-tile ----
                    ypres = []
                    mvs = stpool.tile([128, 4, 2], f32, tag="mvs")
                    for st in range(4):
                        op1 = ppsum.tile([128, CHUNK], f32, tag="ps")
                        op2 = ppsum.tile([128, CHUNK], f32, tag="ps")
                        for dt in range(DT):
                            a_blk = att[:, dt, st * 128:(st + 1) * 128]
                            nc.tensor.matmul(op1[:, 0:512], a_blk,
                                             wo_sb[:, dt, 0:512],
                                             start=(dt == 0), stop=False)
                            nc.tensor.matmul(op2[:, 0:256], a_blk,
                                             wo_sb[:, dt, 512:D],
                                             start=(dt == 0), stop=False)
                        nc.tensor.matmul(op1[:, 0:512], ones_bf[:],
                                         bo_bf[:, 0:512], start=False, stop=True)
                        nc.tensor.matmul(op2[:, 0:256], ones_bf[:],
                                         bo_bf[:, 512:D], start=False, stop=True)
                        ypre = yppool.tile([128, D], f32, tag="ypre")
                        nc.vector.tensor_tensor(ypre[:, 0:512], op1[:, 0:512],
                                                x_sb[:, st, 0:512], ALU.add)
                        nc.vector.tensor_tensor(ypre[:, 512:D], op2[:, 0:256],
                                                x_sb[:, st, 512:D], ALU.add)
                        st6 = stpool.tile([128, 2, 6], f32, tag="st6")
                        ypre_g = ypre[:].rearrange("p (g f) -> p g f", g=2)
                        nc.vector.bn_stats(st6[:, 0, :], ypre_g[:, 0, :])
                        nc.vector.bn_stats(st6[:, 1, :], ypre_g[:, 1, :])
                        nc.vector.bn_aggr(mvs[:, st, :], st6[:])
                        ypres.append(ypre)

                    # ---- batched rstd (one Ln + one Exp per chunk) ----
                    lnv = stpool.tile([128, 4], f32, tag="lnv")
                    rstds = stpool.tile([128, 4], f32, tag="rstds")
                    nmr = stpool.tile([128, 4], f32, tag="nmr")
                    nc.scalar.activation(lnv[:], mvs[:, :, 1], AF.Ln,
                                         bias=eps_sb[:])
                    nc.scalar.activation(rstds[:], lnv[:], AF.Exp, scale=-0.5)
                    nc.vector.tensor_tensor(nmr[:], mvs[:, :, 0], rstds[:],
                                            ALU.mult)
                    nc.vector.tensor_scalar_mul(nmr[:], nmr[:], -1.0)

                    for st in range(4):
                        y1 = ypool.tile([128, D], bf16, tag="y1")
                        nc.scalar.activation(y1[:], ypres[st][:], AF.Identity,
                                             bias=nmr[:, st:st + 1],
                                             scale=rstds[:, st:st + 1])
                        y2 = ypool.tile([128, D], bf16, tag="y2")
                        y3 = ypool.tile([128, D], f32, tag="y3")
                        nc.gpsimd.tensor_tensor(y2[:], y1[:], gb_bc[:],
                                                ALU.mult)
                        nc.gpsimd.tensor_tensor(y3[:], y2[:], be_bc[:], ALU.add)
                        nc.gpsimd.dma_start(
                            out_ap[s0 + st * 128:s0 + (st + 1) * 128, :], y3[:])

                # Pipeline: scores-half0 of chunk c-1 needs no right halo, so it
                # is emitted before project(c); half1 (which reads the halo
                # columns filled by project(c)) comes after. AV halves trail
                # project so their broadcast DMAs have a completion window.
                pend = None
                x_tiles = [None] * NCH
                x_tiles[0] = load_x(0)
                for c in range(NCH):
                    if c + 1 < NCH:
                        x_tiles[c + 1] = load_x(c + 1)
                    x_sb = x_tiles[c]
                    if pend is not None:
                        px, pq, pst = pend
                        attn_sm(c - 1, pq, pst, 0)
                        attn_av(c - 1, pst, 0)
                    qT = project(c, x_sb)
                    if pend is not None:
                        attn_sm(c - 1, pq, pst, 1)
                        attn_av(c - 1, pst, 1)
                        attn_oln(c - 1, px, pst)
                    pend = (x_sb, qT, attn_begin(c))
                px, pq, pst = pend
                attn_sm(NCH - 1, pq, pst, 0)
                attn_av(NCH - 1, pst, 0)
                attn_sm(NCH - 1, pq, pst, 1)
                attn_av(NCH - 1, pst, 1)
                attn_oln(NCH - 1, px, pst)

    nc.compile()
    return nc


def kernel(**inputs):
    if "nc" not in _cache:
        _cache["nc"] = _build()
    nc = _cache["nc"]
    from concourse.bass_utils import run_bass_kernel_spmd

    names = ["Wq", "bq", "Wk", "bk", "Wv", "bv", "Wo", "bo", "gamma", "beta"]
    shared = {n: np.ascontiguousarray(np.asarray(inputs[n], dtype=np.float32))
              for n in names}
    x = np.asarray(inputs["x"], dtype=np.float32)
    in_maps = [dict(shared, x=np.ascontiguousarray(x[b])) for b in range(N_CORES)]
    res = run_bass_kernel_spmd(nc, in_maps, core_ids=list(range(N_CORES)))
    out = np.stack([res.results[i]["out"] for i in range(N_CORES)], axis=0)
    return out.astype(np.float32)

